# revision 1
# baseline (speedup 1.0000x reference)
"""Trainium2 Bass kernel for causal GQA self-attention (S=2048, D=4096, H=32,
HKV=8, DH=128), tensor-parallel over 8 NeuronCores.

Sharding: head-parallel TP. Core i owns q-heads [4i..4i+4) and kv-head i:
  - qkv_proj column shard  -> q [S,512], k [S,128], v [S,128]
  - RoPE + causal attention for its 4 heads (GQA group shares the kv head)
  - o_proj row shard (rows [512i..512i+512)) -> fp32 partial [S, D]
Host sums the 8 partials (the "all-reduce") and reshapes to [S, 1, D].

Layouts on device (per core):
  hidT  [D, S]    bf16  hidden transposed (replicated to all cores)
  wqk   [D, 640]  bf16  q cols (512) ++ k col block (128)
  wv    [D, 128]  bf16
  wo    [512, D]  bf16  o_proj row shard
  cosT/sinT [64, S] bf16 RoPE tables (dh-major)
  out   [S, D]    f32   partial output

All matmuls run in bf16 with fp32 PSUM accumulation. Softmax runs without
max-subtraction (logits are O(10) for this problem's N(0,1)-scale data, far
inside fp32 exp range), which lets ctx accumulate directly in PSUM.
"""

import sys

sys.path.insert(0, "/opt/trn_rl_repo")

import numpy as np
import ml_dtypes
from contextlib import ExitStack

import concourse.bass as bass
import concourse.tile as tile
from concourse import mybir
from concourse.bass_utils import run_bass_kernel_spmd
from concourse.masks import make_causal_mask, make_identity

S, B, D = 2048, 1, 4096
H, HKV, DH = 32, 8, 128
NCORES = 8
HQ = H // HKV  # q heads per core = 4
THETA = 10000.0
SCALE = 1.0 / float(np.sqrt(DH))

BF16 = mybir.dt.bfloat16
F32 = mybir.dt.float32
np_bf16 = ml_dtypes.bfloat16

NKB = D // 128  # 32 contraction blocks for the projections
NQB = S // 128  # 16 query blocks
NCHUNK = S // 512  # 4 sequence chunks of 512


def build_kernel() -> bass.Bass:
    nc = bass.Bass()

    hidT_e = nc.declare_dram_parameter("hidT", [D, S], BF16, isOutput=False)
    wqk_e = nc.declare_dram_parameter("wqk", [D, (HQ + 1) * DH], BF16, isOutput=False)
    wv_e = nc.declare_dram_parameter("wv", [D, DH], BF16, isOutput=False)
    wo_e = nc.declare_dram_parameter("wo", [HQ * DH, D], BF16, isOutput=False)
    # cos2 = [cos; cos], sinS = [-sin; sin]  (dh-major halves stacked)
    cos_e = nc.declare_dram_parameter("cos2", [128, S], BF16, isOutput=False)
    sin_e = nc.declare_dram_parameter("sinS", [128, S], BF16, isOutput=False)
    out_e = nc.declare_dram_parameter("out", [S, D], F32, isOutput=True)

    hidT = hidT_e[:]
    wqk = wqk_e[:]
    wv = wv_e[:]
    wo = wo_e[:]
    out = out_e[:]

    with tile.TileContext(nc) as tc, ExitStack() as ctx:
        singles = ctx.enter_context(tc.tile_pool(name="singles", bufs=1))

        # ---- persistent SBUF state ----
        wqk_sb = singles.tile([128, NKB, (HQ + 1) * DH], BF16)
        wv_sb = singles.tile([128, NKB, DH], BF16)
        wo_sb = singles.tile([128, HQ, D], BF16)
        cos_sb = singles.tile([128, S], BF16)
        sin_sb = singles.tile([128, S], BF16)
        ident = singles.tile([128, 128], BF16)
        cmask = singles.tile([128, 128], F32)
        # qkT: 5 slabs [dh, S] (4 q heads + the kv head), dh-major
        qkT_sb = singles.tile([128, HQ + 1, S], BF16)
        # V, seq-major: tile t = rows [128t..128t+128) x [dh 128]
        v_sb = singles.tile([128, NQB, DH], BF16)
        # ctxT: per q-head slab [dh, S]
        ctxT_sb = singles.tile([128, HQ, S], BF16)

        make_identity(nc, ident)
        make_causal_mask(nc, cmask, mask_val=-1e9)

        # ---- phase 1: qkv projections ----
        with (
            tc.tile_pool(name="hidp", bufs=16) as hidp,
            tc.tile_pool(name="ropep", bufs=6) as ropep,
            tc.tile_pool(name="qk_ps_pool", bufs=6, space="PSUM") as qkpp,
            tc.tile_pool(name="v_ps_pool", bufs=2, space="PSUM") as vpp,
        ):
            for n in range(NCHUNK):
                qk_ps = [
                    qkpp.tile([128, 512], F32, name=f"qk_ps_{n}_{m}", tag="qk_ps")
                    for m in range(HQ + 1)
                ]
                v_ps = vpp.tile([128, 512], F32, name=f"v_ps_{n}", tag="v_ps")
                for kb in range(NKB):
                    if n == 0:
                        # first use of this kb's weight tiles: load them here so
                        # the first matmuls only wait for the loads they need
                        nc.sync.dma_start(
                            out=wqk_sb[:, kb, :], in_=wqk[kb * 128:(kb + 1) * 128, :]
                        )
                        nc.sync.dma_start(
                            out=wv_sb[:, kb, :], in_=wv[kb * 128:(kb + 1) * 128, :]
                        )
                        if kb == 2:
                            nc.sync.dma_start(out=cos_sb, in_=cos_e[:])
                            nc.sync.dma_start(out=sin_sb, in_=sin_e[:])
                    ht = hidp.tile([128, 512], BF16, name="ht", tag="ht")
                    nc.sync.dma_start(
                        out=ht,
                        in_=hidT[kb * 128:(kb + 1) * 128, n * 512:(n + 1) * 512],
                    )
                    first, last = kb == 0, kb == NKB - 1
                    for m in range(HQ + 1):
                        nc.tensor.matmul(
                            qk_ps[m],
                            wqk_sb[:, kb, m * 128:(m + 1) * 128],
                            ht,
                            start=first,
                            stop=last,
                        )
                    for sub in range(4):
                        # one accumulation group for the whole bank: start only
                        # on the first matmul touching it, stop on the last
                        # (start=True lazily zeroes the full 2KB zero region)
                        nc.tensor.matmul(
                            v_ps[:, sub * 128:(sub + 1) * 128],
                            ht[:, sub * 128:(sub + 1) * 128],
                            wv_sb[:, kb, :],
                            start=first and sub == 0,
                            stop=last and sub == 3,
                        )
                for m in range(HQ + 1):
                    nc.scalar.copy(qkT_sb[:, m, n * 512:(n + 1) * 512], qk_ps[m])
                nc.vector.tensor_copy(
                    v_sb[:, n * 4:(n + 1) * 4, :],
                    v_ps.rearrange("p (t d) -> p t d", t=4),
                )
                # RoPE this chunk of each slab right away (k-slab first) so
                # attention on early q-chunks can start while later projection
                # chunks are still running
                sl = slice(n * 512, (n + 1) * 512)
                for m in [HQ] + list(range(HQ)):
                    rot = ropep.tile([128, 512], BF16, name="rope_rot", tag="rot")
                    nc.sync.dma_start(out=rot[0:64, :], in_=qkT_sb[64:128, m, sl])
                    nc.sync.dma_start(out=rot[64:128, :], in_=qkT_sb[0:64, m, sl])
                    rt = ropep.tile([128, 512], BF16, name="rope_rt", tag="rt")
                    nc.vector.tensor_mul(rt, rot, sin_sb[:, sl])
                    nc.vector.tensor_mul(
                        qkT_sb[:, m, sl], qkT_sb[:, m, sl], cos_sb[:, sl]
                    )
                    nc.vector.tensor_add(qkT_sb[:, m, sl], qkT_sb[:, m, sl], rt)
                if n in (1, 2):
                    # o_proj weights, not needed until attention finishes chunk 0
                    for h in (n - 1) * 2, (n - 1) * 2 + 1:
                        nc.sync.dma_start(
                            out=wo_sb[:, h, :], in_=wo[h * 128:(h + 1) * 128, :]
                        )

        # ---- phase 2+3: attention + o_proj, per 512-wide q chunk ----
        with (
            tc.tile_pool(name="p_pool", bufs=3) as pp,
            tc.tile_pool(name="pt_pool", bufs=1) as ptp,
            tc.tile_pool(name="l_pool", bufs=6) as lp,
            tc.tile_pool(name="s_ps_pool", bufs=3, space="PSUM") as spp,
            tc.tile_pool(name="tp_ps_pool", bufs=2, space="PSUM") as tpp,
            tc.tile_pool(name="ctx_ps_pool", bufs=1, space="PSUM") as cpp,
            tc.tile_pool(name="out_ps_pool", bufs=2, space="PSUM") as opp,
            tc.tile_pool(name="out_sb_pool", bufs=6) as osp,
        ):
            for c in range(NCHUNK):
                ntile = 4 * (c + 1)  # kv tiles needed by this q chunk
                for h in range(HQ):
                    pt_sb = ptp.tile(
                        [128, ntile, 512], BF16, name=f"pt_{c}_{h}", tag="pt"
                    )
                    for iq in range(4):
                        qb = 4 * c + iq
                        kmax = (qb + 1) * 128
                        nchunks = (kmax + 511) // 512
                        p_sb = pp.tile([128, kmax], BF16, name="p_sb", tag="p_sb")
                        l_acc = lp.tile([128, 1], F32, name="l_acc", tag="l_acc")
                        nc.vector.memset(l_acc, 0.0)
                        qT = qkT_sb[:, h, qb * 128:(qb + 1) * 128]
                        for j in range(nchunks):
                            w = min(512, kmax - j * 512)
                            s_ps = spp.tile([128, 512], F32, name="s_ps", tag="s_ps")
                            nc.tensor.matmul(
                                s_ps[:, :w],
                                qT,
                                qkT_sb[:, HQ, j * 512:j * 512 + w],
                                start=True,
                                stop=True,
                            )
                            if j == nchunks - 1:
                                nc.vector.tensor_add(
                                    s_ps[:, w - 128:w], s_ps[:, w - 128:w], cmask
                                )
                            lpart = lp.tile([128, 1], F32, name="lpart", tag="lpart")
                            nc.scalar.activation(
                                p_sb[:, j * 512:j * 512 + w],
                                s_ps[:, :w],
                                mybir.ActivationFunctionType.Exp,
                                scale=SCALE,
                                accum_out=lpart,
                            )
                            nc.vector.tensor_add(l_acc, l_acc, lpart)
                        linv = lp.tile([128, 1], F32, name="linv", tag="linv")
                        nc.vector.reciprocal(linv, l_acc)
                        nc.vector.tensor_scalar_mul(p_sb, p_sb, linv)
                        # transpose the normalized P into pt_sb[:, t, iq*128:...]
                        for t in range(qb + 1):
                            pt_ps = tpp.tile([128, 128], BF16, name="pt_ps", tag="pt_ps")
                            nc.tensor.transpose(
                                pt_ps, p_sb[:, t * 128:(t + 1) * 128], ident
                            )
                            nc.vector.tensor_copy(
                                pt_sb[:, t, iq * 128:(iq + 1) * 128], pt_ps
                            )
                    # PV: ctxT[dh, 512q] accumulated over kv tiles
                    ctx_ps = cpp.tile([128, 512], F32, name="ctx_ps", tag="ctx_ps")
                    for t in range(ntile):
                        if t < 4 * c:
                            nc.tensor.matmul(
                                ctx_ps,
                                v_sb[:, t, :],
                                pt_sb[:, t, :],
                                start=(t == 0),
                                stop=False,
                            )
                        else:
                            for iq in range(t - 4 * c, 4):
                                nc.tensor.matmul(
                                    ctx_ps[:, iq * 128:(iq + 1) * 128],
                                    v_sb[:, t, :],
                                    pt_sb[:, t, iq * 128:(iq + 1) * 128],
                                    start=(t == 0 and iq == 0),
                                    stop=(t == ntile - 1 and iq == 3),
                                )
                    nc.scalar.copy(ctxT_sb[:, h, c * 512:(c + 1) * 512], ctx_ps)

                # o_proj for this chunk's 4 query blocks
                for iq in range(4):
                    qb = 4 * c + iq
                    for dc in range(8):
                        out_ps = opp.tile([128, 512], F32, name="out_ps", tag="out_ps")
                        for h in range(HQ):
                            nc.tensor.matmul(
                                out_ps,
                                ctxT_sb[:, h, qb * 128:(qb + 1) * 128],
                                wo_sb[:, h, dc * 512:(dc + 1) * 512],
                                start=(h == 0),
                                stop=(h == HQ - 1),
                            )
                        out_sb = osp.tile([128, 512], F32, name="out_sb", tag="out_sb")
                        if dc % 2 == 0:
                            nc.scalar.copy(out_sb, out_ps)
                        else:
                            nc.vector.tensor_copy(out_sb, out_ps)
                        nc.sync.dma_start(
                            out=out[qb * 128:(qb + 1) * 128, dc * 512:(dc + 1) * 512],
                            in_=out_sb,
                        )

    return nc


def _legalize_waits(j):
    """Split multi-wait instructions: the TPB ISA gives each instruction (and
    each dynamic-DMA descriptor) a single semaphore-wait slot, and this walrus
    build errors on extras instead of splitting them. Hoist all but one wait
    into standalone EventSemaphore instructions on the issuing engine, placed
    immediately before the instruction (engine streams execute in program
    order, so the waits complete before the op issues / the descriptor posts).
    """
    n_new = 0
    for fn in j["functions"]:
        for bb in fn["blocks"]:
            insts = bb.get("instructions", [])
            out = []
            for inst in insts:
                si = inst.get("sync_info") or {}
                waits = si.get("on_wait") or []
                if len(waits) > 1:
                    for w in waits[:-1]:
                        n_new += 1
                        out.append(
                            {
                                "name": f"{inst['name']}-lw{n_new}",
                                "opcode": "EventSemaphore",
                                "engine": inst["engine"],
                                "ins": [],
                                "outs": [],
                                "debug": inst.get("debug"),
                                "sync_info": {"on_update": [], "on_wait": [w]},
                            }
                        )
                    si = dict(si)
                    si["on_wait"] = [waits[-1]]
                    inst = dict(inst)
                    inst["sync_info"] = si
                out.append(inst)
            bb["instructions"] = out
    return j


def _patch_json(nc):
    import json

    orig = nc.to_json_bytes

    def patched():
        j = json.loads(orig())
        return json.dumps(_legalize_waits(j)).encode()

    nc.to_json_bytes = patched
    return nc


_NC_CACHE = None


def _get_nc():
    global _NC_CACHE
    if _NC_CACHE is None:
        _NC_CACHE = _patch_json(build_kernel())
    return _NC_CACHE


def _prep_in_maps(hidden_states, W_qkv, W_o):
    hid = np.asarray(hidden_states, dtype=np.float32).reshape(S, D)
    hidT = np.ascontiguousarray(hid.T).astype(np_bf16)
    W_qkv = np.asarray(W_qkv, dtype=np.float32)
    W_o = np.asarray(W_o, dtype=np.float32)

    inv = 1.0 / (THETA ** (np.arange(0, DH, 2, dtype=np.float64) / DH))
    fr = np.arange(S, dtype=np.float64)[:, None] * inv[None, :]  # [S, 64]
    cosT = np.cos(fr).T
    sinT = np.sin(fr).T
    cos2 = np.ascontiguousarray(np.concatenate([cosT, cosT], 0)).astype(np_bf16)
    sinS = np.ascontiguousarray(np.concatenate([-sinT, sinT], 0)).astype(np_bf16)

    in_maps = []
    for i in range(NCORES):
        q_cols = W_qkv[:, 512 * i:512 * i + 512]
        k_cols = W_qkv[:, H * DH + 128 * i:H * DH + 128 * i + 128]
        v_cols = W_qkv[:, (H + HKV) * DH + 128 * i:(H + HKV) * DH + 128 * i + 128]
        wqk_i = np.ascontiguousarray(
            np.concatenate([q_cols, k_cols], axis=1)
        ).astype(np_bf16)
        wv_i = np.ascontiguousarray(v_cols).astype(np_bf16)
        wo_i = np.ascontiguousarray(W_o[512 * i:512 * i + 512, :]).astype(np_bf16)
        in_maps.append(
            {
                "hidT": hidT,
                "wqk": wqk_i,
                "wv": wv_i,
                "wo": wo_i,
                "cos2": cos2,
                "sinS": sinS,
            }
        )
    return in_maps


def _run(in_maps, trace=False, **kw):
    nc = _get_nc()
    return run_bass_kernel_spmd(
        nc, in_maps, core_ids=list(range(NCORES)), trace=trace, **kw
    )


def _gather(res):
    total = np.zeros((S, D), dtype=np.float32)
    for i in range(NCORES):
        total += np.asarray(res.results[i]["out"], dtype=np.float32)
    return total.reshape(S, B, D).astype(np.float32)


def kernel(hidden_states, sequence_mask, W_qkv, W_o):
    in_maps = _prep_in_maps(hidden_states, W_qkv, W_o)
    return _gather(_run(in_maps))

